# revision 4
# baseline (speedup 1.0000x reference)
"""Bass/Trainium2 kernel for GQA attention (B=1, LQ=LK=2048, D=4096,
H=32, KVH=8, DH=128) distributed over 8 NeuronCores, tensor-parallel by
heads: core i owns kv-head i and its 4 query heads.

Per-core pipeline (all matmuls bf16, accumulation fp32 in PSUM):
  1. qT/kT/v projections from host-transposed hidden states
  2. scoresT = kT . qT per 128k x 512q block (causal blocks only),
     bias+mask added on DVE, exp on ACT
  3. U_T  += v . eT      (unnormalized attention output, transposed)
     S_bc += ones . eT   (row sums broadcast over partitions)
     out_head = U_T * reciprocal(S_bc)
  4. partial_out = attnT . Wo_shard ; host sums the 8 partials
"""
import os
import sys
import types

import numpy as np
import ml_dtypes

sys.path.insert(0, '/opt/trn_rl_repo')

BF16 = ml_dtypes.bfloat16

# ---------------------------------------------------------------- axon shim
def _install_axon_hooks():
    """Provide antenv.axon_hooks (absent in this image) so that
    run_bass_kernel_spmd(trace=True) / BASS_TRACE=1 can capture NTFF
    profiles instead of crashing on import."""
    if "antenv.axon_hooks" in sys.modules:
        return
    state = {"hook": None}
    mod = types.ModuleType("antenv.axon_hooks")
    mod.set_axon_ntff_profile_hook = lambda h: state.__setitem__("hook", h)
    mod.get_axon_ntff_profile_hook = lambda: state["hook"]
    sys.modules["antenv.axon_hooks"] = mod
    try:
        from trn_agent_boot.trn_boot import _ntff_profile_via_ctypes
        mod.set_axon_ntff_profile_hook(
            _ntff_profile_via_ctypes('/opt/axon/libaxon_pjrt.so'))
    except Exception:
        pass


_install_axon_hooks()

import concourse.bass as bass
import concourse.tile as tile
from concourse import mybir
from concourse.bass_utils import run_bass_kernel_spmd
from concourse.alu_op_type import AluOpType

# ---------------------------------------------------------------- constants
B, LQ, LK = 1, 2048, 2048
D, H, KVH, DH = 4096, 32, 8, 128
G = H // KVH          # 4 query heads per kv head
N_CORES = 8
NH = H // N_CORES     # 4 heads per core
KO = D // 128         # 32 contraction chunks for the projections
QC = 512              # q free-dim chunk for attention blocks
NEG = -30000.0        # additive mask value (exp -> exactly 0 in fp32)

FP32 = mybir.dt.float32
DT = mybir.dt.bfloat16


def _split_drain_tile_context():
    """TileContext whose final drain splits its semaphore waits across
    multiple drain instructions — walrus in this container rejects CTRL
    instructions carrying more than one sync wait."""
    import bass_rust

    class SplitDrainTC(tile.TileContext):
        def _drain_and_barrier(self, tick_clock, wait_clock):
            drain_inst = self.nc.sync.drain()
            wait_clock.add_sem_waits(
                drain_inst.ins, tile.ScopedClock({None: tick_clock.global_clock})
            )
            si = drain_inst.ins.sync_info
            if si is not None and si.on_wait and len(si.on_wait) > 1:
                waits = list(si.on_wait)
                si.on_wait = waits[:1]
                drain_inst.ins.sync_info = si
                for w in waits[1:]:
                    d2 = self.nc.sync.drain()
                    d2.ins.sync_info = bass_rust.SyncInfo(on_wait=[w], on_update=[])

            self.nc.all_engine_barrier()
            assert self.sems is not None
            popped = self.nc._tile_sem_poison_stack.pop()
            assert popped is self._sem_poison
            self.nc.clear_and_free_semaphores(list(self.sems.allocated().values()))
            self.nc.all_engine_barrier()

    return SplitDrainTC


def build_graph(nk_per_qc):
    """Build the single-core SPMD graph. nk_per_qc[qc] = number of 128-wide
    key chunks to process for query chunk qc (derived from the mask)."""
    nc = bass.Bass("TRN2", target_bir_lowering=False, debug=False,
                   num_devices=N_CORES)

    hqT = nc.dram_tensor("hqT", [D, LQ], DT, kind="ExternalInput").ap()
    hkvT = nc.dram_tensor("hkvT", [D, LK], DT, kind="ExternalInput").ap()
    wq = nc.dram_tensor("wq", [D, NH * DH], DT, kind="ExternalInput").ap()
    wk = nc.dram_tensor("wk", [D, DH], DT, kind="ExternalInput").ap()
    wv = nc.dram_tensor("wv", [D, DH], DT, kind="ExternalInput").ap()
    wo = nc.dram_tensor("wo", [NH * DH, D], DT, kind="ExternalInput").ap()
    biasT = nc.dram_tensor("biasT", [NH, LK, LQ], DT, kind="ExternalInput").ap()
    out = nc.dram_tensor("out", [LQ, D], FP32, kind="ExternalOutput").ap()

    n_s = LQ // QC        # 4 query chunks of 512
    n_m = LQ // 128       # 16 seq chunks of 128

    TC = _split_drain_tile_context()
    with TC(nc) as tc:
        with tc.tile_pool(name="weights", bufs=1) as wpool, \
             tc.tile_pool(name="persist", bufs=1) as ppool:
            # resident weights
            wq_sb = wpool.tile([128, KO, NH * DH], DT)
            for g in range(4):
                nc.gpsimd.dma_start(
                    out=wq_sb[:, g * 8:(g + 1) * 8, :],
                    in_=wq[g * 1024:(g + 1) * 1024, :].rearrange(
                        "(ko p) d -> p ko d", p=128))
            wk_sb = wpool.tile([128, KO, DH], DT)
            nc.gpsimd.dma_start(
                out=wk_sb[:], in_=wk.rearrange("(ko p) d -> p ko d", p=128))
            wv_sb = wpool.tile([128, KO, DH], DT)
            nc.gpsimd.dma_start(
                out=wv_sb[:], in_=wv.rearrange("(ko p) d -> p ko d", p=128))
            wo_sb = wpool.tile([128, NH, D], DT)
            nc.gpsimd.dma_start(
                out=wo_sb[:], in_=wo.rearrange("(h p) d -> p h d", p=128))
            ones_sb = wpool.tile([128, 128], DT)
            nc.vector.memset(ones_sb[:], 1.0)

            # persistent activations
            qT_sb = ppool.tile([128, NH, LQ], DT)     # [dh, h, q]
            kT_sb = ppool.tile([128, LK], DT)         # [dh, k]
            v_sb = ppool.tile([128, LK // 128, DH], DT)   # [k_in, k_blk, dh]
            un_sb = ppool.tile([128, NH, LQ], DT)     # normalized U_T

            # ---------------- stage 1: projections ----------------
            with tc.tile_pool(name="slab", bufs=2) as slab_pool, \
                 tc.tile_pool(name="proj_ps", bufs=2, space="PSUM") as proj_ps, \
                 tc.tile_pool(name="vproj_ps", bufs=2, space="PSUM") as vproj_ps:
                # kv side
                for s in range(n_s):
                    slab = slab_pool.tile([128, KO, QC], DT, tag="slab")
                    for g in range(4):
                        nc.gpsimd.dma_start(
                            out=slab[:, g * 8:(g + 1) * 8, :],
                            in_=hkvT[g * 1024:(g + 1) * 1024,
                                     s * QC:(s + 1) * QC].rearrange(
                                "(ko p) q -> p ko q", p=128))
                    kt_ps = proj_ps.tile([128, QC], FP32, tag="pps")
                    for ko in range(KO):
                        nc.tensor.matmul(kt_ps[:], lhsT=wk_sb[:, ko, :],
                                         rhs=slab[:, ko, :],
                                         start=(ko == 0), stop=(ko == KO - 1))
                    nc.scalar.copy(out=kT_sb[:, s * QC:(s + 1) * QC], in_=kt_ps[:])
                    for sub in range(QC // 128):
                        blk = s * (QC // 128) + sub
                        v_ps = vproj_ps.tile([128, DH], FP32, tag="vps")
                        for ko in range(KO):
                            nc.tensor.matmul(
                                v_ps[:],
                                lhsT=slab[:, ko, sub * 128:(sub + 1) * 128],
                                rhs=wv_sb[:, ko, :],
                                start=(ko == 0), stop=(ko == KO - 1))
                        nc.scalar.copy(out=v_sb[:, blk, :], in_=v_ps[:])
                # q side
                for s in range(n_s):
                    slab = slab_pool.tile([128, KO, QC], DT, tag="slab")
                    for g in range(4):
                        nc.gpsimd.dma_start(
                            out=slab[:, g * 8:(g + 1) * 8, :],
                            in_=hqT[g * 1024:(g + 1) * 1024,
                                    s * QC:(s + 1) * QC].rearrange(
                                "(ko p) q -> p ko q", p=128))
                    for h in range(NH):
                        q_ps = proj_ps.tile([128, QC], FP32, tag="pps")
                        for ko in range(KO):
                            nc.tensor.matmul(
                                q_ps[:],
                                lhsT=wq_sb[:, ko, h * DH:(h + 1) * DH],
                                rhs=slab[:, ko, :],
                                start=(ko == 0), stop=(ko == KO - 1))
                        nc.scalar.copy(out=qT_sb[:, h, s * QC:(s + 1) * QC],
                                       in_=q_ps[:])

            # ---------------- stage 2: attention ----------------
            inv2 = float(DH ** -0.5)
            with tc.tile_pool(name="bias", bufs=2) as bias_pool, \
                 tc.tile_pool(name="att_sb", bufs=3) as att_sb, \
                 tc.tile_pool(name="sc_ps", bufs=2, space="PSUM") as sc_pool, \
                 tc.tile_pool(name="acc_ps", bufs=2, space="PSUM") as acc_pool:
                for h in range(NH):
                    for qc in range(n_s):
                        nk = nk_per_qc[qc]
                        bias_sb = bias_pool.tile([128, LK // 128, QC], DT,
                                                 tag="bias")
                        for g in range((nk + 3) // 4):
                            k0, k1 = g * 4, min(nk, g * 4 + 4)
                            nc.gpsimd.dma_start(
                                out=bias_sb[:, k0:k1, :],
                                in_=biasT[h, k0 * 128:k1 * 128,
                                          qc * QC:(qc + 1) * QC].rearrange(
                                    "(ko p) q -> p ko q", p=128))
                        u_ps = acc_pool.tile([128, QC], FP32, tag="ups")
                        s_ps = acc_pool.tile([128, QC], FP32, tag="sps")
                        for kc in range(nk):
                            sc_ps = sc_pool.tile([128, QC], FP32, tag="scps")
                            nc.tensor.matmul(
                                sc_ps[:],
                                lhsT=kT_sb[:, kc * 128:(kc + 1) * 128],
                                rhs=qT_sb[:, h, qc * QC:(qc + 1) * QC],
                                start=True, stop=True)
                            t_sb = att_sb.tile([128, QC], FP32, tag="tsb")
                            # t = score * dh^-0.5 + bias
                            nc.vector.scalar_tensor_tensor(
                                out=t_sb[:], in0=sc_ps[:], scalar=inv2,
                                in1=bias_sb[:, kc, :],
                                op0=AluOpType.mult,
                                op1=AluOpType.add)
                            e_sb = att_sb.tile([128, QC], DT, tag="esb")
                            nc.scalar.activation(
                                out=e_sb[:], in_=t_sb[:],
                                func=mybir.ActivationFunctionType.Exp)
                            nc.tensor.matmul(u_ps[:], lhsT=v_sb[:, kc, :],
                                             rhs=e_sb[:],
                                             start=(kc == 0), stop=(kc == nk - 1))
                            nc.tensor.matmul(s_ps[:], lhsT=ones_sb[:],
                                             rhs=e_sb[:],
                                             start=(kc == 0), stop=(kc == nk - 1))
                        sinv_sb = att_sb.tile([128, QC], FP32, tag="sinv")
                        nc.vector.reciprocal(out=sinv_sb[:], in_=s_ps[:])
                        nc.vector.tensor_mul(
                            un_sb[:, h, qc * QC:(qc + 1) * QC],
                            u_ps[:], sinv_sb[:])

            # ---------------- stage 3: output projection ----------------
            with tc.tile_pool(name="osb", bufs=2) as out_pool, \
                 tc.tile_pool(name="ops", bufs=4, space="PSUM") as out_ps:
                for m in range(n_m):
                    o_sb = out_pool.tile([128, D], FP32, tag="osb")
                    for n in range(D // QC):
                        o_ps = out_ps.tile([128, QC], FP32, tag="ops")
                        for h in range(NH):
                            nc.tensor.matmul(
                                o_ps[:],
                                lhsT=un_sb[:, h, m * 128:(m + 1) * 128],
                                rhs=wo_sb[:, h, n * QC:(n + 1) * QC],
                                start=(h == 0), stop=(h == NH - 1))
                        nc.scalar.copy(out=o_sb[:, n * QC:(n + 1) * QC],
                                       in_=o_ps[:])
                    nc.gpsimd.dma_start(out=out[m * 128:(m + 1) * 128, :],
                                        in_=o_sb[:])
    _split_waits(nc)
    return nc


def _split_waits(nc):
    """Walrus in this container accepts at most one sync wait per
    instruction: hoist extra waits onto same-engine nops placed directly
    before the instruction (identical semantics — the engine stream
    blocks on each in order)."""
    import bass_rust
    ctr = 0
    for f in nc.m.functions:
        for bb in f.blocks:
            new = []
            for inst in bb.instructions:
                si = inst.sync_info
                if si is not None and si.on_wait and len(si.on_wait) > 1:
                    waits = list(si.on_wait)
                    for w in waits[:-1]:
                        nop = bass_rust.InstNoOp(name=f"waitnop-{ctr}",
                                                 engine=inst.engine)
                        ctr += 1
                        nop.sync_info = bass_rust.SyncInfo(on_wait=[w],
                                                           on_update=[])
                        new.append(nop)
                    si.on_wait = waits[-1:]
                    inst.sync_info = si
                new.append(inst)
            bb.instructions = new


_CACHE = {}


def kernel(hidden_q, hidden_kv, attention_mask, position_bias, Wq, Wk, Wv, Wo):
    hq = np.asarray(hidden_q, dtype=np.float32)[0]      # [2048, 4096]
    hkv = np.asarray(hidden_kv, dtype=np.float32)[0]
    mask = np.asarray(attention_mask)[0]                # [2048, 2048] bool
    pb = np.asarray(position_bias, dtype=np.float32)    # [32, 2048, 2048]
    Wq = np.asarray(Wq, dtype=np.float32)
    Wk = np.asarray(Wk, dtype=np.float32)
    Wv = np.asarray(Wv, dtype=np.float32)
    Wo = np.asarray(Wo, dtype=np.float32)

    # additive mask, transposed to [k, q]
    negT = np.where(mask, np.float32(0.0), np.float32(NEG)).T

    # which 128-key chunks are live for each 512-query chunk
    n_s = LQ // QC
    nk_per_qc = []
    for qc in range(n_s):
        cols = negT[:, qc * QC:(qc + 1) * QC]            # [2048k, 512q]
        live = 0
        for kc in range(LK // 128):
            if np.any(cols[kc * 128:(kc + 1) * 128] != np.float32(NEG)):
                live = kc + 1
        nk_per_qc.append(live)
    key = tuple(nk_per_qc)

    if key not in _CACHE:
        _CACHE[key] = build_graph(nk_per_qc)
    nc = _CACHE[key]

    hqT = np.ascontiguousarray(hq.T).astype(BF16)        # [4096, 2048]
    hkvT = np.ascontiguousarray(hkv.T).astype(BF16)

    in_maps = []
    for i in range(N_CORES):
        bT = np.transpose(pb[NH * i:NH * (i + 1)], (0, 2, 1))  # [4, k, q]
        biasT = (bT + negT[None]).astype(BF16)
        in_maps.append({
            "hqT": hqT,
            "hkvT": hkvT,
            "wq": np.ascontiguousarray(Wq[:, i * NH * DH:(i + 1) * NH * DH]).astype(BF16),
            "wk": np.ascontiguousarray(Wk[:, i * DH:(i + 1) * DH]).astype(BF16),
            "wv": np.ascontiguousarray(Wv[:, i * DH:(i + 1) * DH]).astype(BF16),
            "wo": np.ascontiguousarray(Wo[i * NH * DH:(i + 1) * NH * DH, :]).astype(BF16),
            "biasT": biasT,
        })

    res = run_bass_kernel_spmd(nc, in_maps, list(range(N_CORES)))
    kernel.last_results = res

    acc = np.zeros((LQ, D), dtype=np.float32)
    for i in range(N_CORES):
        acc += res.results[i]["out"]
    return acc[None]


# revision 11
# speedup vs baseline: 1.0039x; 1.0039x over previous
"""Bass/Trainium2 kernel for GQA attention (B=1, LQ=LK=2048, D=4096,
H=32, KVH=8, DH=128) distributed over 8 NeuronCores, tensor-parallel by
heads: core i owns kv-head i and its 4 query heads.

Per-core pipeline (all matmuls bf16, accumulation fp32 in PSUM):
  1. qT/kT/v projections from host-transposed hidden states
  2. scoresT = kT . qT per 128k x 512q block (causal blocks only),
     bias+mask added on DVE, exp on ACT
  3. U_T  += v . eT      (unnormalized attention output, transposed)
     S_bc += ones . eT   (row sums broadcast over partitions)
     out_head = U_T * reciprocal(S_bc)
  4. partial_out = attnT . Wo_shard ; host sums the 8 partials
"""
import os
import sys
import types

import numpy as np
import ml_dtypes

sys.path.insert(0, '/opt/trn_rl_repo')

BF16 = ml_dtypes.bfloat16

# ---------------------------------------------------------------- axon shim
def _install_axon_hooks():
    """Provide antenv.axon_hooks (absent in this image) so that
    run_bass_kernel_spmd(trace=True) / BASS_TRACE=1 can capture NTFF
    profiles instead of crashing on import."""
    if "antenv.axon_hooks" in sys.modules:
        return
    state = {"hook": None}
    mod = types.ModuleType("antenv.axon_hooks")
    mod.set_axon_ntff_profile_hook = lambda h: state.__setitem__("hook", h)
    mod.get_axon_ntff_profile_hook = lambda: state["hook"]
    sys.modules["antenv.axon_hooks"] = mod
    try:
        from trn_agent_boot.trn_boot import _ntff_profile_via_ctypes
        mod.set_axon_ntff_profile_hook(
            _ntff_profile_via_ctypes('/opt/axon/libaxon_pjrt.so'))
    except Exception:
        pass


_install_axon_hooks()

import concourse.bass as bass
import concourse.tile as tile
from concourse import mybir
from concourse.bass_utils import run_bass_kernel_spmd
from concourse.alu_op_type import AluOpType
from concourse.masks import make_identity

# ---------------------------------------------------------------- constants
B, LQ, LK = 1, 2048, 2048
D, H, KVH, DH = 4096, 32, 8, 128
G = H // KVH          # 4 query heads per kv head
N_CORES = 8
NH = H // N_CORES     # 4 heads per core
KO = D // 128         # 32 contraction chunks for the projections
QC = 512              # q free-dim chunk for attention blocks
NEG = -30000.0        # additive mask value (exp -> exactly 0 in fp32)

FP32 = mybir.dt.float32
DT = mybir.dt.bfloat16


def _split_drain_tile_context():
    """TileContext whose final drain splits its semaphore waits across
    multiple drain instructions — walrus in this container rejects CTRL
    instructions carrying more than one sync wait."""
    import bass_rust

    class SplitDrainTC(tile.TileContext):
        def _drain_and_barrier(self, tick_clock, wait_clock):
            drain_inst = self.nc.sync.drain()
            wait_clock.add_sem_waits(
                drain_inst.ins, tile.ScopedClock({None: tick_clock.global_clock})
            )
            si = drain_inst.ins.sync_info
            if si is not None and si.on_wait and len(si.on_wait) > 1:
                waits = list(si.on_wait)
                si.on_wait = waits[:1]
                drain_inst.ins.sync_info = si
                for w in waits[1:]:
                    d2 = self.nc.sync.drain()
                    d2.ins.sync_info = bass_rust.SyncInfo(on_wait=[w], on_update=[])

            self.nc.all_engine_barrier()
            assert self.sems is not None
            popped = self.nc._tile_sem_poison_stack.pop()
            assert popped is self._sem_poison
            self.nc.clear_and_free_semaphores(list(self.sems.allocated().values()))
            self.nc.all_engine_barrier()

    return SplitDrainTC


def build_graph(nk_per_qc):
    """Build the single-core SPMD graph. nk_per_qc[qc] = number of 128-wide
    key chunks to process for query chunk qc (derived from the mask)."""
    nc = bass.Bass("TRN2", target_bir_lowering=False, debug=False,
                   num_devices=N_CORES)

    hqT = nc.dram_tensor("hqT", [D, LQ], DT, kind="ExternalInput").ap()
    hkvT = nc.dram_tensor("hkvT", [D, LK], DT, kind="ExternalInput").ap()
    wq = nc.dram_tensor("wq", [D, NH * DH], DT, kind="ExternalInput").ap()
    wk = nc.dram_tensor("wk", [D, DH], DT, kind="ExternalInput").ap()
    wv = nc.dram_tensor("wv", [D, DH], DT, kind="ExternalInput").ap()
    wo = nc.dram_tensor("wo", [NH * DH, D], DT, kind="ExternalInput").ap()
    biasT = nc.dram_tensor("biasT", [NH, LK, LQ], DT, kind="ExternalInput").ap()
    out = nc.dram_tensor("out", [LQ, D], FP32, kind="ExternalOutput").ap()

    n_s = LQ // QC        # 4 query chunks of 512
    n_m = LQ // 128       # 16 seq chunks of 128

    TC = _split_drain_tile_context()
    with TC(nc) as tc:
        with tc.tile_pool(name="weights", bufs=1) as wpool, \
             tc.tile_pool(name="persist", bufs=1) as ppool:
            # kv-side weights first (HWDGE ring) so the first matmuls can
            # start as soon as the first hkvT slab lands
            wk_sb = wpool.tile([128, KO, DH], DT)
            nc.sync.dma_start(
                out=wk_sb[:], in_=wk.rearrange("(ko p) d -> p ko d", p=128))
            wv_sb = wpool.tile([128, KO, DH], DT)
            nc.sync.dma_start(
                out=wv_sb[:], in_=wv.rearrange("(ko p) d -> p ko d", p=128))
            ones_sb = wpool.tile([128, 128], DT)
            nc.vector.memset(ones_sb[:], 1.0)
            ident_sb = wpool.tile([128, 128], DT)
            make_identity(nc, ident_sb[:])

            # persistent activations
            qT_sb = ppool.tile([128, NH, LQ], DT)     # [dh, h, q]
            kT_sb = ppool.tile([128, LK], DT)         # [dh, k]
            vT_sb = ppool.tile([128, LK], DT)         # [dh, k]
            v_sb = ppool.tile([128, LK // 128, DH], DT)   # [k_in, k_blk, dh]
            un_sb = ppool.tile([128, NH, LQ], DT)     # normalized U_T

            # ---------------- stage 1: projections ----------------
            with tc.tile_pool(name="slab", bufs=2) as slab_pool, \
                 tc.tile_pool(name="proj_ps", bufs=2, space="PSUM") as proj_ps, \
                 tc.tile_pool(name="vtr_ps", bufs=2, space="PSUM") as vtr_ps, \
                 nc.named_scope("proj"):
                # kv side
                for s in range(n_s):
                    slab = slab_pool.tile([128, KO, QC], DT, tag="slab")
                    for g in range(4):
                        nc.gpsimd.dma_start(
                            out=slab[:, g * 8:(g + 1) * 8, :],
                            in_=hkvT[g * 1024:(g + 1) * 1024,
                                     s * QC:(s + 1) * QC].rearrange(
                                "(ko p) q -> p ko q", p=128))
                    kt_ps = proj_ps.tile([128, QC], FP32, tag="pps")
                    for ko in range(KO):
                        nc.tensor.matmul(kt_ps[:], lhsT=wk_sb[:, ko, :],
                                         rhs=slab[:, ko, :],
                                         start=(ko == 0), stop=(ko == KO - 1))
                    nc.scalar.copy(out=kT_sb[:, s * QC:(s + 1) * QC], in_=kt_ps[:])
                    vt_ps = proj_ps.tile([128, QC], FP32, tag="pps")
                    for ko in range(KO):
                        nc.tensor.matmul(vt_ps[:], lhsT=wv_sb[:, ko, :],
                                         rhs=slab[:, ko, :],
                                         start=(ko == 0), stop=(ko == KO - 1))
                    nc.scalar.copy(out=vT_sb[:, s * QC:(s + 1) * QC], in_=vt_ps[:])
                # v natural layout via PE transpose of vT
                for blk in range(LK // 128):
                    tp = vtr_ps.tile([128, 128], DT, tag="vtr")
                    nc.tensor.transpose(
                        tp[:], vT_sb[:, blk * 128:(blk + 1) * 128], ident_sb[:])
                    nc.scalar.copy(out=v_sb[:, blk, :], in_=tp[:])
                # q side
                wq_sb = wpool.tile([128, KO, NH * DH], DT)
                for g in range(4):
                    nc.sync.dma_start(
                        out=wq_sb[:, g * 8:(g + 1) * 8, :],
                        in_=wq[g * 1024:(g + 1) * 1024, :].rearrange(
                            "(ko p) d -> p ko d", p=128))
                for s in range(n_s):
                    slab = slab_pool.tile([128, KO, QC], DT, tag="slab")
                    for g in range(4):
                        nc.gpsimd.dma_start(
                            out=slab[:, g * 8:(g + 1) * 8, :],
                            in_=hqT[g * 1024:(g + 1) * 1024,
                                    s * QC:(s + 1) * QC].rearrange(
                                "(ko p) q -> p ko q", p=128))
                    for h in range(NH):
                        q_ps = proj_ps.tile([128, QC], FP32, tag="pps")
                        for ko in range(KO):
                            nc.tensor.matmul(
                                q_ps[:],
                                lhsT=wq_sb[:, ko, h * DH:(h + 1) * DH],
                                rhs=slab[:, ko, :],
                                start=(ko == 0), stop=(ko == KO - 1))
                        nc.scalar.copy(out=qT_sb[:, h, s * QC:(s + 1) * QC],
                                       in_=q_ps[:])

            # ---------------- stage 2: attention ----------------
            inv2 = float(DH ** -0.5)
            with tc.tile_pool(name="bias", bufs=2) as bias_pool, \
                 tc.tile_pool(name="att_sb", bufs=4) as att_sb, \
                 tc.tile_pool(name="sc_ps", bufs=4, space="PSUM") as sc_pool, \
                 tc.tile_pool(name="acc_ps", bufs=2, space="PSUM") as acc_pool, \
                 nc.named_scope("attn"):
                for h in range(NH):
                    for qc in range(n_s):
                        nk = nk_per_qc[qc]
                        bias_sb = bias_pool.tile([128, LK // 128, QC], DT,
                                                 tag="bias")
                        for g in range((nk + 3) // 4):
                            k0, k1 = g * 4, min(nk, g * 4 + 4)
                            nc.gpsimd.dma_start(
                                out=bias_sb[:, k0:k1, :],
                                in_=biasT[h, k0 * 128:k1 * 128,
                                          qc * QC:(qc + 1) * QC].rearrange(
                                    "(ko p) q -> p ko q", p=128))
                        u_ps = acc_pool.tile([128, QC], FP32, tag="ups")
                        s_ps = acc_pool.tile([128, QC], FP32, tag="sps")
                        e_tiles = {}
                        # software pipeline: emit score/bias/exp for kc before
                        # the accumulation matmuls of kc-1 so the PE stream has
                        # score work to chew on while DVE/ACT produce e(kc)
                        for kc in range(nk + 1):
                            if kc < nk:
                                sc_ps = sc_pool.tile([128, QC], FP32, tag="scps")
                                nc.tensor.matmul(
                                    sc_ps[:],
                                    lhsT=kT_sb[:, kc * 128:(kc + 1) * 128],
                                    rhs=qT_sb[:, h, qc * QC:(qc + 1) * QC],
                                    start=True, stop=True)
                                t_sb = att_sb.tile([128, QC], FP32, tag="tsb")
                                # t = score * dh^-0.5 + bias
                                nc.vector.scalar_tensor_tensor(
                                    out=t_sb[:], in0=sc_ps[:], scalar=inv2,
                                    in1=bias_sb[:, kc, :],
                                    op0=AluOpType.mult,
                                    op1=AluOpType.add)
                                e_sb = att_sb.tile([128, QC], DT, tag="esb")
                                nc.scalar.activation(
                                    out=e_sb[:], in_=t_sb[:],
                                    func=mybir.ActivationFunctionType.Exp)
                                e_tiles[kc] = e_sb
                            if kc >= 1:
                                e_prev = e_tiles.pop(kc - 1)
                                nc.tensor.matmul(u_ps[:], lhsT=v_sb[:, kc - 1, :],
                                                 rhs=e_prev[:],
                                                 start=(kc == 1),
                                                 stop=(kc == nk))
                                nc.tensor.matmul(s_ps[:], lhsT=ones_sb[:],
                                                 rhs=e_prev[:],
                                                 start=(kc == 1),
                                                 stop=(kc == nk))
                        sinv_sb = att_sb.tile([128, QC], FP32, tag="sinv")
                        nc.vector.reciprocal(out=sinv_sb[:], in_=s_ps[:])
                        nc.vector.tensor_mul(
                            un_sb[:, h, qc * QC:(qc + 1) * QC],
                            u_ps[:], sinv_sb[:])

            # ---------------- stage 3: output projection ----------------
            wo_sb = wpool.tile([128, NH, D], DT)
            nc.sync.dma_start(
                out=wo_sb[:], in_=wo.rearrange("(h p) d -> p h d", p=128))
            with tc.tile_pool(name="osb", bufs=2) as out_pool, \
                 tc.tile_pool(name="ops", bufs=8, space="PSUM") as out_ps, \
                 nc.named_scope("wo"):
                for m in range(n_m):
                    o_sb = out_pool.tile([128, D], FP32, tag="osb")
                    # n-chunks in halves with h outer so each un_sb lhsT is
                    # loaded once per half (4 LDWEIGHTS per half, not 16)
                    for half in range(2):
                        hps = [out_ps.tile([128, QC], FP32, tag="ops",
                                           name=f"ops_{m}_{half}_{j}")
                               for j in range(4)]
                        for h in range(NH):
                            for j in range(4):
                                n = half * 4 + j
                                nc.tensor.matmul(
                                    hps[j][:],
                                    lhsT=un_sb[:, h, m * 128:(m + 1) * 128],
                                    rhs=wo_sb[:, h, n * QC:(n + 1) * QC],
                                    start=(h == 0), stop=(h == NH - 1))
                        for j in range(4):
                            n = half * 4 + j
                            nc.scalar.copy(out=o_sb[:, n * QC:(n + 1) * QC],
                                           in_=hps[j][:])
                    nc.gpsimd.dma_start(out=out[m * 128:(m + 1) * 128, :],
                                        in_=o_sb[:])
    _split_waits(nc)
    return nc


def _split_waits(nc):
    """Walrus in this container accepts at most one sync wait per
    instruction: hoist extra waits onto same-engine nops placed directly
    before the instruction (identical semantics — the engine stream
    blocks on each in order)."""
    import bass_rust
    ctr = 0
    for f in nc.m.functions:
        for bb in f.blocks:
            new = []
            for inst in bb.instructions:
                si = inst.sync_info
                if si is not None and si.on_wait and len(si.on_wait) > 1:
                    waits = list(si.on_wait)
                    for w in waits[:-1]:
                        nop = bass_rust.InstNoOp(name=f"waitnop-{ctr}",
                                                 engine=inst.engine)
                        ctr += 1
                        nop.sync_info = bass_rust.SyncInfo(on_wait=[w],
                                                           on_update=[])
                        new.append(nop)
                    si.on_wait = waits[-1:]
                    inst.sync_info = si
                new.append(inst)
            bb.instructions = new


_CACHE = {}


def kernel(hidden_q, hidden_kv, attention_mask, position_bias, Wq, Wk, Wv, Wo):
    hq = np.asarray(hidden_q, dtype=np.float32)[0]      # [2048, 4096]
    hkv = np.asarray(hidden_kv, dtype=np.float32)[0]
    mask = np.asarray(attention_mask)[0]                # [2048, 2048] bool
    pb = np.asarray(position_bias, dtype=np.float32)    # [32, 2048, 2048]
    Wq = np.asarray(Wq, dtype=np.float32)
    Wk = np.asarray(Wk, dtype=np.float32)
    Wv = np.asarray(Wv, dtype=np.float32)
    Wo = np.asarray(Wo, dtype=np.float32)

    # additive mask, transposed to [k, q]
    negT = np.where(mask, np.float32(0.0), np.float32(NEG)).T

    # which 128-key chunks are live for each 512-query chunk
    n_s = LQ // QC
    nk_per_qc = []
    for qc in range(n_s):
        cols = negT[:, qc * QC:(qc + 1) * QC]            # [2048k, 512q]
        live = 0
        for kc in range(LK // 128):
            if np.any(cols[kc * 128:(kc + 1) * 128] != np.float32(NEG)):
                live = kc + 1
        nk_per_qc.append(live)
    key = tuple(nk_per_qc)

    if key not in _CACHE:
        _CACHE[key] = build_graph(nk_per_qc)
    nc = _CACHE[key]

    hqT = np.ascontiguousarray(hq.T).astype(BF16)        # [4096, 2048]
    hkvT = np.ascontiguousarray(hkv.T).astype(BF16)

    in_maps = []
    for i in range(N_CORES):
        bT = np.transpose(pb[NH * i:NH * (i + 1)], (0, 2, 1))  # [4, k, q]
        biasT = (bT + negT[None]).astype(BF16)
        in_maps.append({
            "hqT": hqT,
            "hkvT": hkvT,
            "wq": np.ascontiguousarray(Wq[:, i * NH * DH:(i + 1) * NH * DH]).astype(BF16),
            "wk": np.ascontiguousarray(Wk[:, i * DH:(i + 1) * DH]).astype(BF16),
            "wv": np.ascontiguousarray(Wv[:, i * DH:(i + 1) * DH]).astype(BF16),
            "wo": np.ascontiguousarray(Wo[i * NH * DH:(i + 1) * NH * DH, :]).astype(BF16),
            "biasT": biasT,
        })

    res = run_bass_kernel_spmd(nc, in_maps, list(range(N_CORES)))
    kernel.last_results = res

    acc = np.zeros((LQ, D), dtype=np.float32)
    for i in range(N_CORES):
        acc += res.results[i]["out"]
    return acc[None]


# revision 17
# speedup vs baseline: 1.0448x; 1.0408x over previous
"""Bass/Trainium2 kernel for GQA attention (B=1, LQ=LK=2048, D=4096,
H=32, KVH=8, DH=128) distributed over 8 NeuronCores, tensor-parallel by
heads: core i owns kv-head i and its 4 query heads.

Per-core pipeline (all matmuls bf16, accumulation fp32 in PSUM):
  1. qT/kT/v projections from host-transposed hidden states
  2. scoresT = kT . qT per 128k x 512q block (causal blocks only),
     bias+mask added on DVE, exp on ACT
  3. U_T  += v . eT      (unnormalized attention output, transposed)
     S_bc += ones . eT   (row sums broadcast over partitions)
     out_head = U_T * reciprocal(S_bc)
  4. partial_out = attnT . Wo_shard ; host sums the 8 partials
"""
import os
import sys
import types

import numpy as np
import ml_dtypes

sys.path.insert(0, '/opt/trn_rl_repo')

BF16 = ml_dtypes.bfloat16

# ---------------------------------------------------------------- axon shim
def _install_axon_hooks():
    """Provide antenv.axon_hooks (absent in this image) so that
    run_bass_kernel_spmd(trace=True) / BASS_TRACE=1 can capture NTFF
    profiles instead of crashing on import."""
    if "antenv.axon_hooks" in sys.modules:
        return
    state = {"hook": None}
    mod = types.ModuleType("antenv.axon_hooks")
    mod.set_axon_ntff_profile_hook = lambda h: state.__setitem__("hook", h)
    mod.get_axon_ntff_profile_hook = lambda: state["hook"]
    sys.modules["antenv.axon_hooks"] = mod
    try:
        from trn_agent_boot.trn_boot import _ntff_profile_via_ctypes
        mod.set_axon_ntff_profile_hook(
            _ntff_profile_via_ctypes('/opt/axon/libaxon_pjrt.so'))
    except Exception:
        pass


_install_axon_hooks()

import concourse.bass as bass
import concourse.tile as tile
from concourse import mybir
from concourse.bass_utils import run_bass_kernel_spmd
from concourse.alu_op_type import AluOpType
from concourse.masks import make_identity

# ---------------------------------------------------------------- constants
B, LQ, LK = 1, 2048, 2048
D, H, KVH, DH = 4096, 32, 8, 128
G = H // KVH          # 4 query heads per kv head
N_CORES = 8
NH = H // N_CORES     # 4 heads per core
KO = D // 128         # 32 contraction chunks for the projections
QC = 512              # q free-dim chunk for attention blocks
NEG = -30000.0        # additive mask value (exp -> exactly 0 in fp32)

FP32 = mybir.dt.float32
DT = mybir.dt.bfloat16


def _split_drain_tile_context():
    """TileContext whose final drain splits its semaphore waits across
    multiple drain instructions — walrus in this container rejects CTRL
    instructions carrying more than one sync wait."""
    import bass_rust

    class SplitDrainTC(tile.TileContext):
        def _drain_and_barrier(self, tick_clock, wait_clock):
            drain_inst = self.nc.sync.drain()
            wait_clock.add_sem_waits(
                drain_inst.ins, tile.ScopedClock({None: tick_clock.global_clock})
            )
            si = drain_inst.ins.sync_info
            if si is not None and si.on_wait and len(si.on_wait) > 1:
                waits = list(si.on_wait)
                si.on_wait = waits[:1]
                drain_inst.ins.sync_info = si
                for w in waits[1:]:
                    d2 = self.nc.sync.drain()
                    d2.ins.sync_info = bass_rust.SyncInfo(on_wait=[w], on_update=[])

            self.nc.all_engine_barrier()
            assert self.sems is not None
            popped = self.nc._tile_sem_poison_stack.pop()
            assert popped is self._sem_poison
            self.nc.clear_and_free_semaphores(list(self.sems.allocated().values()))
            self.nc.all_engine_barrier()

    return SplitDrainTC


def build_graph(nk_per_qc):
    """Build the single-core SPMD graph. nk_per_qc[qc] = number of 128-wide
    key chunks to process for query chunk qc (derived from the mask)."""
    nc = bass.Bass("TRN2", target_bir_lowering=False, debug=False,
                   num_devices=N_CORES)

    hqT = nc.dram_tensor("hqT", [D, LQ], DT, kind="ExternalInput").ap()
    hkvT = nc.dram_tensor("hkvT", [D, LK], DT, kind="ExternalInput").ap()
    wq = nc.dram_tensor("wq", [D, NH * DH], DT, kind="ExternalInput").ap()
    wk = nc.dram_tensor("wk", [D, DH], DT, kind="ExternalInput").ap()
    wv = nc.dram_tensor("wv", [D, DH], DT, kind="ExternalInput").ap()
    wo = nc.dram_tensor("wo", [NH * DH, D], DT, kind="ExternalInput").ap()
    biasT = nc.dram_tensor("biasT", [NH, LK, LQ], DT, kind="ExternalInput").ap()
    out = nc.dram_tensor("out", [LQ, D], DT, kind="ExternalOutput").ap()

    n_s = LQ // QC        # 4 query chunks of 512
    n_m = LQ // 128       # 16 seq chunks of 128

    TC = _split_drain_tile_context()
    with TC(nc) as tc:
        with tc.tile_pool(name="weights", bufs=1) as wpool, \
             tc.tile_pool(name="persist", bufs=1) as ppool:
            ones_sb = wpool.tile([128, 128], DT)
            nc.vector.memset(ones_sb[:], 1.0)
            ident_sb = wpool.tile([128, 128], DT)
            make_identity(nc, ident_sb[:])
            # preload the exp table set while projections run
            warm_sb = wpool.tile([128, 1], FP32)
            nc.scalar.activation(out=warm_sb[:], in_=ones_sb[:, 0:1],
                                 func=mybir.ActivationFunctionType.Exp)

            # persistent activations
            qT_sb = ppool.tile([128, NH, LQ], DT)     # [dh, h, q]
            kT_sb = ppool.tile([128, LK], DT)         # [dh, k]
            v_sb = ppool.tile([128, LK // 128, DH], DT)   # [k_in, k_blk, dh]
            un_sb = ppool.tile([128, NH, LQ], DT)     # normalized U_T

            # ---------------- stage 1: projections ----------------
            with tc.tile_pool(name="w1", bufs=1) as w1pool, \
                 tc.tile_pool(name="slab", bufs=2) as slab_pool, \
                 tc.tile_pool(name="proj_ps", bufs=2, space="PSUM") as proj_ps, \
                 tc.tile_pool(name="vtr_ps", bufs=2, space="PSUM") as vtr_ps, \
                 nc.named_scope("proj"):
                # kv-side weights first (HWDGE ring) so the first matmuls
                # can start as soon as the first hkvT slab lands
                wk_sb = w1pool.tile([128, KO, DH], DT)
                nc.sync.dma_start(
                    out=wk_sb[:], in_=wk.rearrange("(ko p) d -> p ko d", p=128))
                wv_sb = w1pool.tile([128, KO, DH], DT)
                nc.sync.dma_start(
                    out=wv_sb[:], in_=wv.rearrange("(ko p) d -> p ko d", p=128))
                vT_sb = w1pool.tile([128, LK], DT)        # [dh, k]
                # kv side: half-size slabs (16 of 32 ko chunks each) to fit
                # SBUF; kT/vT psums accumulate across both halves
                for s in range(n_s):
                    kt_ps = proj_ps.tile([128, QC], FP32, tag="pps",
                                         name=f"ktps_{s}")
                    vt_ps = proj_ps.tile([128, QC], FP32, tag="pps2",
                                         name=f"vtps_{s}")
                    for half in range(2):
                        slab = slab_pool.tile([128, KO // 2, QC], DT,
                                              tag="slab", name=f"kvslab_{s}_{half}")
                        for g in range(2):
                            r0 = half * 2048 + g * 1024
                            nc.gpsimd.dma_start(
                                out=slab[:, g * 8:(g + 1) * 8, :],
                                in_=hkvT[r0:r0 + 1024,
                                         s * QC:(s + 1) * QC].rearrange(
                                    "(ko p) q -> p ko q", p=128))
                        for kl in range(KO // 2):
                            ko = half * (KO // 2) + kl
                            nc.tensor.matmul(kt_ps[:], lhsT=wk_sb[:, ko, :],
                                             rhs=slab[:, kl, :],
                                             start=(ko == 0), stop=(ko == KO - 1))
                        for kl in range(KO // 2):
                            ko = half * (KO // 2) + kl
                            nc.tensor.matmul(vt_ps[:], lhsT=wv_sb[:, ko, :],
                                             rhs=slab[:, kl, :],
                                             start=(ko == 0), stop=(ko == KO - 1))
                    nc.scalar.copy(out=kT_sb[:, s * QC:(s + 1) * QC], in_=kt_ps[:])
                    nc.scalar.copy(out=vT_sb[:, s * QC:(s + 1) * QC], in_=vt_ps[:])
                # v natural layout via PE transpose of vT
                for blk in range(LK // 128):
                    tp = vtr_ps.tile([128, 128], DT, tag="vtr")
                    nc.tensor.transpose(
                        tp[:], vT_sb[:, blk * 128:(blk + 1) * 128], ident_sb[:])
                    nc.scalar.copy(out=v_sb[:, blk, :], in_=tp[:])
                # q side
                wq_sb = w1pool.tile([128, KO, NH * DH], DT)
                for g in range(4):
                    nc.sync.dma_start(
                        out=wq_sb[:, g * 8:(g + 1) * 8, :],
                        in_=wq[g * 1024:(g + 1) * 1024, :].rearrange(
                            "(ko p) d -> p ko d", p=128))
                for s in range(n_s):
                    q_pss = [proj_ps.tile([128, QC], FP32,
                                          tag=("pps" if h % 2 == 0 else "pps2"),
                                          name=f"qps_{s}_{h}")
                             for h in range(NH)]
                    for half in range(2):
                        slab = slab_pool.tile([128, KO // 2, QC], DT,
                                              tag="slab", name=f"qslab_{s}_{half}")
                        for g in range(2):
                            r0 = half * 2048 + g * 1024
                            nc.gpsimd.dma_start(
                                out=slab[:, g * 8:(g + 1) * 8, :],
                                in_=hqT[r0:r0 + 1024,
                                        s * QC:(s + 1) * QC].rearrange(
                                    "(ko p) q -> p ko q", p=128))
                        for h in range(NH):
                            for kl in range(KO // 2):
                                ko = half * (KO // 2) + kl
                                nc.tensor.matmul(
                                    q_pss[h][:],
                                    lhsT=wq_sb[:, ko, h * DH:(h + 1) * DH],
                                    rhs=slab[:, kl, :],
                                    start=(ko == 0), stop=(ko == KO - 1))
                    for h in range(NH):
                        nc.scalar.copy(out=qT_sb[:, h, s * QC:(s + 1) * QC],
                                       in_=q_pss[h][:])

            # ------- stage 2+3: attention interleaved with out-proj -------
            # unnormalized U_T and row-sums staged to SBUF by ACT so the
            # reciprocal/normalize never sits on the DVE critical path
            wo_sb = wpool.tile([128, NH, D], DT)
            nc.sync.dma_start(
                out=wo_sb[:], in_=wo.rearrange("(h p) d -> p h d", p=128))

            inv2 = float(DH ** -0.5)
            with tc.tile_pool(name="bias", bufs=2) as bias_pool, \
                 tc.tile_pool(name="att_sb", bufs=4) as att_sb, \
                 tc.tile_pool(name="us_sb", bufs=6) as us_pool, \
                 tc.tile_pool(name="sc_ps", bufs=2, space="PSUM") as sc_pool, \
                 tc.tile_pool(name="acc_ps", bufs=2, space="PSUM") as acc_pool, \
                 tc.tile_pool(name="osb", bufs=2) as out_pool, \
                 tc.tile_pool(name="ops", bufs=2, space="PSUM") as out_ps, \
                 nc.named_scope("attn_wo"):

                def emit_attn_unit(qc, h):
                    nk = nk_per_qc[qc]
                    bias_sb = bias_pool.tile([128, LK // 128, QC], DT,
                                             tag="bias", name=f"bias_{qc}_{h}")
                    for g in range((nk + 3) // 4):
                        k0, k1 = g * 4, min(nk, g * 4 + 4)
                        nc.gpsimd.dma_start(
                            out=bias_sb[:, k0:k1, :],
                            in_=biasT[h, k0 * 128:k1 * 128,
                                      qc * QC:(qc + 1) * QC].rearrange(
                                "(ko p) q -> p ko q", p=128))
                    u_ps = acc_pool.tile([128, QC], FP32, tag="ups",
                                         name=f"ups_{qc}_{h}")
                    s_ps = acc_pool.tile([128, QC], FP32, tag="sps",
                                         name=f"sps_{qc}_{h}")
                    e_tiles = {}
                    # software pipeline: score/bias/exp for kc emitted before
                    # the accumulation matmuls of kc-1
                    for kc in range(nk + 1):
                        if kc < nk:
                            sc_ps = sc_pool.tile([128, QC], FP32, tag="scps",
                                                 name=f"scps_{qc}_{h}_{kc}")
                            nc.tensor.matmul(
                                sc_ps[:],
                                lhsT=kT_sb[:, kc * 128:(kc + 1) * 128],
                                rhs=qT_sb[:, h, qc * QC:(qc + 1) * QC],
                                start=True, stop=True)
                            t_sb = att_sb.tile([128, QC], FP32, tag="tsb",
                                               name=f"tsb_{qc}_{h}_{kc}")
                            # t = score * dh^-0.5 + bias
                            nc.vector.scalar_tensor_tensor(
                                out=t_sb[:], in0=sc_ps[:], scalar=inv2,
                                in1=bias_sb[:, kc, :],
                                op0=AluOpType.mult,
                                op1=AluOpType.add)
                            e_sb = att_sb.tile([128, QC], DT, tag="esb",
                                               name=f"esb_{qc}_{h}_{kc}")
                            nc.scalar.activation(
                                out=e_sb[:], in_=t_sb[:],
                                func=mybir.ActivationFunctionType.Exp)
                            e_tiles[kc] = e_sb
                        if kc >= 1:
                            e_prev = e_tiles.pop(kc - 1)
                            nc.tensor.matmul(u_ps[:], lhsT=v_sb[:, kc - 1, :],
                                             rhs=e_prev[:],
                                             start=(kc == 1), stop=(kc == nk))
                            nc.tensor.matmul(s_ps[:], lhsT=ones_sb[:],
                                             rhs=e_prev[:],
                                             start=(kc == 1), stop=(kc == nk))
                    u_sb = us_pool.tile([128, QC], DT, tag="usb",
                                        name=f"usb_{qc}_{h}")
                    s_sb = us_pool.tile([128, QC], DT, tag="ssb",
                                        name=f"ssb_{qc}_{h}")
                    nc.scalar.copy(out=u_sb[:], in_=u_ps[:])
                    nc.scalar.copy(out=s_sb[:], in_=s_ps[:])
                    us_tiles[(qc, h)] = (u_sb, s_sb)

                def emit_norm(qc):
                    qsl = slice(qc * QC, (qc + 1) * QC)
                    for h in range(NH):
                        u_sb, s_sb = us_tiles.pop((qc, h))
                        sinv_sb = att_sb.tile([128, QC], DT, tag="sinv",
                                              name=f"sinv_{qc}_{h}")
                        with nc.allow_low_precision(
                                reason="softmax denom, bf16 ulp is plenty"):
                            nc.vector.reciprocal(out=sinv_sb[:], in_=s_sb[:])
                        nc.vector.tensor_mul(
                            un_sb[:, h, qsl], u_sb[:], sinv_sb[:])

                def emit_wo_unit(m):
                    o_sb = out_pool.tile([128, D], DT, tag="osb",
                                         name=f"osb_{m}")
                    for n in range(D // QC):
                        o_ps = out_ps.tile([128, QC], FP32, tag="ops",
                                           name=f"ops_{m}_{n}")
                        for h in range(NH):
                            nc.tensor.matmul(
                                o_ps[:],
                                lhsT=un_sb[:, h, m * 128:(m + 1) * 128],
                                rhs=wo_sb[:, h, n * QC:(n + 1) * QC],
                                start=(h == 0), stop=(h == NH - 1))
                        nc.scalar.copy(out=o_sb[:, n * QC:(n + 1) * QC],
                                       in_=o_ps[:])
                    nc.gpsimd.dma_start(out=out[m * 128:(m + 1) * 128, :],
                                        in_=o_sb[:])

                # interleave: attention units stream; once qc's 4 heads are
                # done, norm(qc) unlocks wo chunks m=4qc..4qc+3, which are
                # woven between the following attention units
                us_tiles = {}
                wo_queue = []
                for h in range(NH):
                    emit_attn_unit(0, h)
                emit_norm(0)
                wo_queue.extend(range(0, 4))
                for qc in range(1, n_s):
                    for h in range(NH):
                        emit_attn_unit(qc, h)
                        if wo_queue:
                            emit_wo_unit(wo_queue.pop(0))
                    emit_norm(qc)
                    wo_queue.extend(range(4 * qc, 4 * qc + 4))
                for m in wo_queue:
                    emit_wo_unit(m)
    _split_waits(nc)
    return nc


def _split_waits(nc):
    """Walrus in this container accepts at most one sync wait per
    instruction: hoist extra waits onto same-engine nops placed directly
    before the instruction (identical semantics — the engine stream
    blocks on each in order)."""
    import bass_rust
    ctr = 0
    for f in nc.m.functions:
        for bb in f.blocks:
            new = []
            for inst in bb.instructions:
                si = inst.sync_info
                if si is not None and si.on_wait and len(si.on_wait) > 1:
                    waits = list(si.on_wait)
                    for w in waits[:-1]:
                        nop = bass_rust.InstNoOp(name=f"waitnop-{ctr}",
                                                 engine=inst.engine)
                        ctr += 1
                        nop.sync_info = bass_rust.SyncInfo(on_wait=[w],
                                                           on_update=[])
                        new.append(nop)
                    si.on_wait = waits[-1:]
                    inst.sync_info = si
                new.append(inst)
            bb.instructions = new


_CACHE = {}


def kernel(hidden_q, hidden_kv, attention_mask, position_bias, Wq, Wk, Wv, Wo):
    hq = np.asarray(hidden_q, dtype=np.float32)[0]      # [2048, 4096]
    hkv = np.asarray(hidden_kv, dtype=np.float32)[0]
    mask = np.asarray(attention_mask)[0]                # [2048, 2048] bool
    pb = np.asarray(position_bias, dtype=np.float32)    # [32, 2048, 2048]
    Wq = np.asarray(Wq, dtype=np.float32)
    Wk = np.asarray(Wk, dtype=np.float32)
    Wv = np.asarray(Wv, dtype=np.float32)
    Wo = np.asarray(Wo, dtype=np.float32)

    # additive mask, transposed to [k, q]
    negT = np.where(mask, np.float32(0.0), np.float32(NEG)).T

    # which 128-key chunks are live for each 512-query chunk
    n_s = LQ // QC
    nk_per_qc = []
    for qc in range(n_s):
        cols = negT[:, qc * QC:(qc + 1) * QC]            # [2048k, 512q]
        live = 0
        for kc in range(LK // 128):
            if np.any(cols[kc * 128:(kc + 1) * 128] != np.float32(NEG)):
                live = kc + 1
        nk_per_qc.append(live)
    key = tuple(nk_per_qc)

    if key not in _CACHE:
        _CACHE[key] = build_graph(nk_per_qc)
    nc = _CACHE[key]

    hqT = np.ascontiguousarray(hq.T).astype(BF16)        # [4096, 2048]
    hkvT = np.ascontiguousarray(hkv.T).astype(BF16)

    in_maps = []
    for i in range(N_CORES):
        bT = np.transpose(pb[NH * i:NH * (i + 1)], (0, 2, 1))  # [4, k, q]
        biasT = (bT + negT[None]).astype(BF16)
        in_maps.append({
            "hqT": hqT,
            "hkvT": hkvT,
            "wq": np.ascontiguousarray(Wq[:, i * NH * DH:(i + 1) * NH * DH]).astype(BF16),
            "wk": np.ascontiguousarray(Wk[:, i * DH:(i + 1) * DH]).astype(BF16),
            "wv": np.ascontiguousarray(Wv[:, i * DH:(i + 1) * DH]).astype(BF16),
            "wo": np.ascontiguousarray(Wo[i * NH * DH:(i + 1) * NH * DH, :]).astype(BF16),
            "biasT": biasT,
        })

    res = run_bass_kernel_spmd(nc, in_maps, list(range(N_CORES)))
    kernel.last_results = res

    acc = np.zeros((LQ, D), dtype=np.float32)
    for i in range(N_CORES):
        acc += res.results[i]["out"].astype(np.float32)
    return acc[None]


# revision 18
# speedup vs baseline: 1.0729x; 1.0269x over previous
"""Bass/Trainium2 kernel for GQA attention (B=1, LQ=LK=2048, D=4096,
H=32, KVH=8, DH=128) distributed over 8 NeuronCores, tensor-parallel by
heads: core i owns kv-head i and its 4 query heads.

Per-core pipeline (all matmuls bf16, accumulation fp32 in PSUM):
  1. qT/kT/v projections from host-transposed hidden states
  2. scoresT = kT . qT per 128k x 512q block (causal blocks only),
     bias+mask added on DVE, exp on ACT
  3. U_T  += v . eT      (unnormalized attention output, transposed)
     S_bc += ones . eT   (row sums broadcast over partitions)
     out_head = U_T * reciprocal(S_bc)
  4. partial_out = attnT . Wo_shard ; host sums the 8 partials
"""
import os
import sys
import types

import numpy as np
import ml_dtypes

sys.path.insert(0, '/opt/trn_rl_repo')

BF16 = ml_dtypes.bfloat16

# ---------------------------------------------------------------- axon shim
def _install_axon_hooks():
    """Provide antenv.axon_hooks (absent in this image) so that
    run_bass_kernel_spmd(trace=True) / BASS_TRACE=1 can capture NTFF
    profiles instead of crashing on import."""
    if "antenv.axon_hooks" in sys.modules:
        return
    state = {"hook": None}
    mod = types.ModuleType("antenv.axon_hooks")
    mod.set_axon_ntff_profile_hook = lambda h: state.__setitem__("hook", h)
    mod.get_axon_ntff_profile_hook = lambda: state["hook"]
    sys.modules["antenv.axon_hooks"] = mod
    try:
        from trn_agent_boot.trn_boot import _ntff_profile_via_ctypes
        mod.set_axon_ntff_profile_hook(
            _ntff_profile_via_ctypes('/opt/axon/libaxon_pjrt.so'))
    except Exception:
        pass


_install_axon_hooks()

import concourse.bass as bass
import concourse.tile as tile
from concourse import mybir
from concourse.bass_utils import run_bass_kernel_spmd
from concourse.alu_op_type import AluOpType
from concourse.masks import make_identity

# ---------------------------------------------------------------- constants
B, LQ, LK = 1, 2048, 2048
D, H, KVH, DH = 4096, 32, 8, 128
G = H // KVH          # 4 query heads per kv head
N_CORES = 8
NH = H // N_CORES     # 4 heads per core
KO = D // 128         # 32 contraction chunks for the projections
QC = 512              # q free-dim chunk for attention blocks
NEG = -30000.0        # additive mask value (exp -> exactly 0 in fp32)

FP32 = mybir.dt.float32
DT = mybir.dt.bfloat16


def _split_drain_tile_context():
    """TileContext whose final drain splits its semaphore waits across
    multiple drain instructions — walrus in this container rejects CTRL
    instructions carrying more than one sync wait."""
    import bass_rust

    class SplitDrainTC(tile.TileContext):
        def _drain_and_barrier(self, tick_clock, wait_clock):
            drain_inst = self.nc.sync.drain()
            wait_clock.add_sem_waits(
                drain_inst.ins, tile.ScopedClock({None: tick_clock.global_clock})
            )
            si = drain_inst.ins.sync_info
            if si is not None and si.on_wait and len(si.on_wait) > 1:
                waits = list(si.on_wait)
                si.on_wait = waits[:1]
                drain_inst.ins.sync_info = si
                for w in waits[1:]:
                    d2 = self.nc.sync.drain()
                    d2.ins.sync_info = bass_rust.SyncInfo(on_wait=[w], on_update=[])

            self.nc.all_engine_barrier()
            assert self.sems is not None
            popped = self.nc._tile_sem_poison_stack.pop()
            assert popped is self._sem_poison
            self.nc.clear_and_free_semaphores(list(self.sems.allocated().values()))
            self.nc.all_engine_barrier()

    return SplitDrainTC


def build_graph(nk_per_qc):
    """Build the single-core SPMD graph. nk_per_qc[qc] = number of 128-wide
    key chunks to process for query chunk qc (derived from the mask)."""
    nc = bass.Bass("TRN2", target_bir_lowering=False, debug=False,
                   num_devices=N_CORES)

    hqT = nc.dram_tensor("hqT", [D, LQ], DT, kind="ExternalInput").ap()
    hkvT = nc.dram_tensor("hkvT", [D, LK], DT, kind="ExternalInput").ap()
    wq = nc.dram_tensor("wq", [D, NH * DH], DT, kind="ExternalInput").ap()
    wk = nc.dram_tensor("wk", [D, DH], DT, kind="ExternalInput").ap()
    wv = nc.dram_tensor("wv", [D, DH], DT, kind="ExternalInput").ap()
    wo = nc.dram_tensor("wo", [NH * DH, D], DT, kind="ExternalInput").ap()
    biasT = nc.dram_tensor("biasT", [NH, LK, LQ], DT, kind="ExternalInput").ap()
    out = nc.dram_tensor("out", [LQ, D], DT, kind="ExternalOutput").ap()

    n_s = LQ // QC        # 4 query chunks of 512
    n_m = LQ // 128       # 16 seq chunks of 128

    TC = _split_drain_tile_context()
    with TC(nc) as tc:
        with tc.tile_pool(name="weights", bufs=1) as wpool, \
             tc.tile_pool(name="persist", bufs=1) as ppool:
            ones_sb = wpool.tile([128, 128], DT)
            nc.vector.memset(ones_sb[:], 1.0)
            ident_sb = wpool.tile([128, 128], DT)
            make_identity(nc, ident_sb[:])
            # preload the exp table set while projections run
            warm_sb = wpool.tile([128, 1], FP32)
            nc.scalar.activation(out=warm_sb[:], in_=ones_sb[:, 0:1],
                                 func=mybir.ActivationFunctionType.Exp)

            # persistent activations
            qT_sb = ppool.tile([128, NH, LQ], DT)     # [dh, h, q]
            kT_sb = ppool.tile([128, LK], DT)         # [dh, k]
            v_sb = ppool.tile([128, LK // 128, DH], DT)   # [k_in, k_blk, dh]
            un_sb = ppool.tile([128, NH, LQ], DT)     # normalized U_T

            # ---------------- stage 1: projections ----------------
            with tc.tile_pool(name="w1", bufs=1) as w1pool, \
                 tc.tile_pool(name="slab", bufs=3) as slab_pool, \
                 tc.tile_pool(name="proj_ps", bufs=2, space="PSUM") as proj_ps, \
                 tc.tile_pool(name="vtr_ps", bufs=2, space="PSUM") as vtr_ps, \
                 nc.named_scope("proj"):
                # kv-side weights first (HWDGE ring) so the first matmuls
                # can start as soon as the first hkvT slab lands
                wk_sb = w1pool.tile([128, KO, DH], DT)
                nc.sync.dma_start(
                    out=wk_sb[:], in_=wk.rearrange("(ko p) d -> p ko d", p=128))
                wv_sb = w1pool.tile([128, KO, DH], DT)
                nc.sync.dma_start(
                    out=wv_sb[:], in_=wv.rearrange("(ko p) d -> p ko d", p=128))
                vT_sb = w1pool.tile([128, LK], DT)        # [dh, k]
                # kv side: half-size slabs (16 of 32 ko chunks each) to fit
                # SBUF; kT/vT psums accumulate across both halves
                for s in range(n_s):
                    kt_ps = proj_ps.tile([128, QC], FP32, tag="pps",
                                         name=f"ktps_{s}")
                    vt_ps = proj_ps.tile([128, QC], FP32, tag="pps2",
                                         name=f"vtps_{s}")
                    for half in range(2):
                        slab = slab_pool.tile([128, KO // 2, QC], DT,
                                              tag="slab", name=f"kvslab_{s}_{half}")
                        for g in range(2):
                            r0 = half * 2048 + g * 1024
                            nc.gpsimd.dma_start(
                                out=slab[:, g * 8:(g + 1) * 8, :],
                                in_=hkvT[r0:r0 + 1024,
                                         s * QC:(s + 1) * QC].rearrange(
                                    "(ko p) q -> p ko q", p=128))
                        for kl in range(KO // 2):
                            ko = half * (KO // 2) + kl
                            nc.tensor.matmul(kt_ps[:], lhsT=wk_sb[:, ko, :],
                                             rhs=slab[:, kl, :],
                                             start=(ko == 0), stop=(ko == KO - 1))
                        for kl in range(KO // 2):
                            ko = half * (KO // 2) + kl
                            nc.tensor.matmul(vt_ps[:], lhsT=wv_sb[:, ko, :],
                                             rhs=slab[:, kl, :],
                                             start=(ko == 0), stop=(ko == KO - 1))
                    nc.scalar.copy(out=kT_sb[:, s * QC:(s + 1) * QC], in_=kt_ps[:])
                    nc.scalar.copy(out=vT_sb[:, s * QC:(s + 1) * QC], in_=vt_ps[:])
                # v natural layout via PE transpose of vT
                for blk in range(LK // 128):
                    tp = vtr_ps.tile([128, 128], DT, tag="vtr")
                    nc.tensor.transpose(
                        tp[:], vT_sb[:, blk * 128:(blk + 1) * 128], ident_sb[:])
                    nc.scalar.copy(out=v_sb[:, blk, :], in_=tp[:])
                # q side
                wq_sb = w1pool.tile([128, KO, NH * DH], DT)
                for g in range(4):
                    nc.sync.dma_start(
                        out=wq_sb[:, g * 8:(g + 1) * 8, :],
                        in_=wq[g * 1024:(g + 1) * 1024, :].rearrange(
                            "(ko p) d -> p ko d", p=128))
                for s in range(n_s):
                    q_pss = [proj_ps.tile([128, QC], FP32,
                                          tag=("pps" if h % 2 == 0 else "pps2"),
                                          name=f"qps_{s}_{h}")
                             for h in range(NH)]
                    for half in range(2):
                        slab = slab_pool.tile([128, KO // 2, QC], DT,
                                              tag="slab", name=f"qslab_{s}_{half}")
                        for g in range(2):
                            r0 = half * 2048 + g * 1024
                            nc.gpsimd.dma_start(
                                out=slab[:, g * 8:(g + 1) * 8, :],
                                in_=hqT[r0:r0 + 1024,
                                        s * QC:(s + 1) * QC].rearrange(
                                    "(ko p) q -> p ko q", p=128))
                        for h in range(NH):
                            for kl in range(KO // 2):
                                ko = half * (KO // 2) + kl
                                nc.tensor.matmul(
                                    q_pss[h][:],
                                    lhsT=wq_sb[:, ko, h * DH:(h + 1) * DH],
                                    rhs=slab[:, kl, :],
                                    start=(ko == 0), stop=(ko == KO - 1))
                    for h in range(NH):
                        nc.scalar.copy(out=qT_sb[:, h, s * QC:(s + 1) * QC],
                                       in_=q_pss[h][:])

            # ------- stage 2+3: attention interleaved with out-proj -------
            # unnormalized U_T and row-sums staged to SBUF by ACT so the
            # reciprocal/normalize never sits on the DVE critical path
            wo_sb = wpool.tile([128, NH, D], DT)
            nc.sync.dma_start(
                out=wo_sb[:], in_=wo.rearrange("(h p) d -> p h d", p=128))

            inv2 = float(DH ** -0.5)
            with tc.tile_pool(name="bias", bufs=3) as bias_pool, \
                 tc.tile_pool(name="att_sb", bufs=4) as att_sb, \
                 tc.tile_pool(name="us_sb", bufs=6) as us_pool, \
                 tc.tile_pool(name="sc_ps", bufs=2, space="PSUM") as sc_pool, \
                 tc.tile_pool(name="acc_ps", bufs=2, space="PSUM") as acc_pool, \
                 tc.tile_pool(name="osb", bufs=2) as out_pool, \
                 tc.tile_pool(name="ops", bufs=2, space="PSUM") as out_ps, \
                 nc.named_scope("attn_wo"):

                def emit_attn_unit(qc, h):
                    nk = nk_per_qc[qc]
                    bias_sb = bias_pool.tile([128, LK // 128, QC], DT,
                                             tag="bias", name=f"bias_{qc}_{h}")
                    for g in range((nk + 3) // 4):
                        k0, k1 = g * 4, min(nk, g * 4 + 4)
                        nc.gpsimd.dma_start(
                            out=bias_sb[:, k0:k1, :],
                            in_=biasT[h, k0 * 128:k1 * 128,
                                      qc * QC:(qc + 1) * QC].rearrange(
                                "(ko p) q -> p ko q", p=128))
                    u_ps = acc_pool.tile([128, QC], FP32, tag="ups",
                                         name=f"ups_{qc}_{h}")
                    s_ps = acc_pool.tile([128, QC], FP32, tag="sps",
                                         name=f"sps_{qc}_{h}")
                    e_tiles = {}
                    # software pipeline: score/bias/exp for kc emitted before
                    # the accumulation matmuls of kc-1
                    for kc in range(nk + 1):
                        if kc < nk:
                            sc_ps = sc_pool.tile([128, QC], FP32, tag="scps",
                                                 name=f"scps_{qc}_{h}_{kc}")
                            nc.tensor.matmul(
                                sc_ps[:],
                                lhsT=kT_sb[:, kc * 128:(kc + 1) * 128],
                                rhs=qT_sb[:, h, qc * QC:(qc + 1) * QC],
                                start=True, stop=True)
                            t_sb = att_sb.tile([128, QC], FP32, tag="tsb",
                                               name=f"tsb_{qc}_{h}_{kc}")
                            # t = score * dh^-0.5 + bias
                            nc.vector.scalar_tensor_tensor(
                                out=t_sb[:], in0=sc_ps[:], scalar=inv2,
                                in1=bias_sb[:, kc, :],
                                op0=AluOpType.mult,
                                op1=AluOpType.add)
                            e_sb = att_sb.tile([128, QC], DT, tag="esb",
                                               name=f"esb_{qc}_{h}_{kc}")
                            nc.scalar.activation(
                                out=e_sb[:], in_=t_sb[:],
                                func=mybir.ActivationFunctionType.Exp)
                            e_tiles[kc] = e_sb
                        if kc >= 1:
                            e_prev = e_tiles.pop(kc - 1)
                            nc.tensor.matmul(u_ps[:], lhsT=v_sb[:, kc - 1, :],
                                             rhs=e_prev[:],
                                             start=(kc == 1), stop=(kc == nk))
                            nc.tensor.matmul(s_ps[:], lhsT=ones_sb[:],
                                             rhs=e_prev[:],
                                             start=(kc == 1), stop=(kc == nk))
                    u_sb = us_pool.tile([128, QC], DT, tag="usb",
                                        name=f"usb_{qc}_{h}")
                    s_sb = us_pool.tile([128, QC], DT, tag="ssb",
                                        name=f"ssb_{qc}_{h}")
                    nc.scalar.copy(out=u_sb[:], in_=u_ps[:])
                    nc.scalar.copy(out=s_sb[:], in_=s_ps[:])
                    us_tiles[(qc, h)] = (u_sb, s_sb)

                def emit_norm_piece(qc, h):
                    qsl = slice(qc * QC, (qc + 1) * QC)
                    u_sb, s_sb = us_tiles.pop((qc, h))
                    sinv_sb = att_sb.tile([128, QC], DT, tag="sinv",
                                          name=f"sinv_{qc}_{h}")
                    with nc.allow_low_precision(
                            reason="softmax denom, bf16 ulp is plenty"):
                        nc.vector.reciprocal(out=sinv_sb[:], in_=s_sb[:])
                    nc.vector.tensor_mul(
                        un_sb[:, h, qsl], u_sb[:], sinv_sb[:])

                def emit_wo_unit(m):
                    o_sb = out_pool.tile([128, D], DT, tag="osb",
                                         name=f"osb_{m}")
                    for n in range(D // QC):
                        o_ps = out_ps.tile([128, QC], FP32, tag="ops",
                                           name=f"ops_{m}_{n}")
                        for h in range(NH):
                            nc.tensor.matmul(
                                o_ps[:],
                                lhsT=un_sb[:, h, m * 128:(m + 1) * 128],
                                rhs=wo_sb[:, h, n * QC:(n + 1) * QC],
                                start=(h == 0), stop=(h == NH - 1))
                        nc.scalar.copy(out=o_sb[:, n * QC:(n + 1) * QC],
                                       in_=o_ps[:])
                    nc.scalar.dma_start(out=out[m * 128:(m + 1) * 128, :],
                                        in_=o_sb[:])

                # interleave: attention units stream; norm pieces for qc are
                # woven between the attention units of qc+1 (keeps the
                # reciprocal off the DVE critical path); wo chunks for qc
                # follow once its norm pieces are all emitted
                us_tiles = {}
                wo_queue = []
                for h in range(NH):
                    emit_attn_unit(0, h)
                for qc in range(1, n_s):
                    for h in range(NH):
                        emit_attn_unit(qc, h)
                        emit_norm_piece(qc - 1, h)
                        if h == NH - 1:
                            wo_queue.extend(range(4 * (qc - 1), 4 * qc))
                        if wo_queue:
                            emit_wo_unit(wo_queue.pop(0))
                for h in range(NH):
                    emit_norm_piece(n_s - 1, h)
                    if wo_queue:
                        emit_wo_unit(wo_queue.pop(0))
                wo_queue.extend(range(4 * (n_s - 1), 4 * n_s))
                for m in wo_queue:
                    emit_wo_unit(m)
    _split_waits(nc)
    return nc


def _split_waits(nc):
    """Walrus in this container accepts at most one sync wait per
    instruction: hoist extra waits onto same-engine nops placed directly
    before the instruction (identical semantics — the engine stream
    blocks on each in order)."""
    import bass_rust
    ctr = 0
    for f in nc.m.functions:
        for bb in f.blocks:
            new = []
            for inst in bb.instructions:
                si = inst.sync_info
                if si is not None and si.on_wait and len(si.on_wait) > 1:
                    waits = list(si.on_wait)
                    for w in waits[:-1]:
                        nop = bass_rust.InstNoOp(name=f"waitnop-{ctr}",
                                                 engine=inst.engine)
                        ctr += 1
                        nop.sync_info = bass_rust.SyncInfo(on_wait=[w],
                                                           on_update=[])
                        new.append(nop)
                    si.on_wait = waits[-1:]
                    inst.sync_info = si
                new.append(inst)
            bb.instructions = new


_CACHE = {}


def kernel(hidden_q, hidden_kv, attention_mask, position_bias, Wq, Wk, Wv, Wo):
    hq = np.asarray(hidden_q, dtype=np.float32)[0]      # [2048, 4096]
    hkv = np.asarray(hidden_kv, dtype=np.float32)[0]
    mask = np.asarray(attention_mask)[0]                # [2048, 2048] bool
    pb = np.asarray(position_bias, dtype=np.float32)    # [32, 2048, 2048]
    Wq = np.asarray(Wq, dtype=np.float32)
    Wk = np.asarray(Wk, dtype=np.float32)
    Wv = np.asarray(Wv, dtype=np.float32)
    Wo = np.asarray(Wo, dtype=np.float32)

    # additive mask, transposed to [k, q]
    negT = np.where(mask, np.float32(0.0), np.float32(NEG)).T

    # which 128-key chunks are live for each 512-query chunk
    n_s = LQ // QC
    nk_per_qc = []
    for qc in range(n_s):
        cols = negT[:, qc * QC:(qc + 1) * QC]            # [2048k, 512q]
        live = 0
        for kc in range(LK // 128):
            if np.any(cols[kc * 128:(kc + 1) * 128] != np.float32(NEG)):
                live = kc + 1
        nk_per_qc.append(live)
    key = tuple(nk_per_qc)

    if key not in _CACHE:
        _CACHE[key] = build_graph(nk_per_qc)
    nc = _CACHE[key]

    hqT = np.ascontiguousarray(hq.T).astype(BF16)        # [4096, 2048]
    hkvT = np.ascontiguousarray(hkv.T).astype(BF16)

    in_maps = []
    for i in range(N_CORES):
        bT = np.transpose(pb[NH * i:NH * (i + 1)], (0, 2, 1))  # [4, k, q]
        biasT = (bT + negT[None]).astype(BF16)
        in_maps.append({
            "hqT": hqT,
            "hkvT": hkvT,
            "wq": np.ascontiguousarray(Wq[:, i * NH * DH:(i + 1) * NH * DH]).astype(BF16),
            "wk": np.ascontiguousarray(Wk[:, i * DH:(i + 1) * DH]).astype(BF16),
            "wv": np.ascontiguousarray(Wv[:, i * DH:(i + 1) * DH]).astype(BF16),
            "wo": np.ascontiguousarray(Wo[i * NH * DH:(i + 1) * NH * DH, :]).astype(BF16),
            "biasT": biasT,
        })

    res = run_bass_kernel_spmd(nc, in_maps, list(range(N_CORES)))
    kernel.last_results = res

    acc = np.zeros((LQ, D), dtype=np.float32)
    for i in range(N_CORES):
        acc += res.results[i]["out"].astype(np.float32)
    return acc[None]


# revision 19
# speedup vs baseline: 1.0807x; 1.0073x over previous
"""Bass/Trainium2 kernel for GQA attention (B=1, LQ=LK=2048, D=4096,
H=32, KVH=8, DH=128) distributed over 8 NeuronCores, tensor-parallel by
heads: core i owns kv-head i and its 4 query heads.

Per-core pipeline (all matmuls bf16, accumulation fp32 in PSUM):
  1. qT/kT/v projections from host-transposed hidden states
  2. scoresT = kT . qT per 128k x 512q block (causal blocks only),
     bias+mask added on DVE, exp on ACT
  3. U_T  += v . eT      (unnormalized attention output, transposed)
     S_bc += ones . eT   (row sums broadcast over partitions)
     out_head = U_T * reciprocal(S_bc)
  4. partial_out = attnT . Wo_shard ; host sums the 8 partials
"""
import os
import sys
import types

import numpy as np
import ml_dtypes

sys.path.insert(0, '/opt/trn_rl_repo')

BF16 = ml_dtypes.bfloat16

# ---------------------------------------------------------------- axon shim
def _install_axon_hooks():
    """Provide antenv.axon_hooks (absent in this image) so that
    run_bass_kernel_spmd(trace=True) / BASS_TRACE=1 can capture NTFF
    profiles instead of crashing on import."""
    if "antenv.axon_hooks" in sys.modules:
        return
    state = {"hook": None}
    mod = types.ModuleType("antenv.axon_hooks")
    mod.set_axon_ntff_profile_hook = lambda h: state.__setitem__("hook", h)
    mod.get_axon_ntff_profile_hook = lambda: state["hook"]
    sys.modules["antenv.axon_hooks"] = mod
    try:
        from trn_agent_boot.trn_boot import _ntff_profile_via_ctypes
        mod.set_axon_ntff_profile_hook(
            _ntff_profile_via_ctypes('/opt/axon/libaxon_pjrt.so'))
    except Exception:
        pass


_install_axon_hooks()

import concourse.bass as bass
import concourse.tile as tile
from concourse import mybir
from concourse.bass_utils import run_bass_kernel_spmd
from concourse.alu_op_type import AluOpType
from concourse.masks import make_identity

# ---------------------------------------------------------------- constants
B, LQ, LK = 1, 2048, 2048
D, H, KVH, DH = 4096, 32, 8, 128
G = H // KVH          # 4 query heads per kv head
N_CORES = 8
NH = H // N_CORES     # 4 heads per core
KO = D // 128         # 32 contraction chunks for the projections
QC = 512              # q free-dim chunk for attention blocks
NEG = -30000.0        # additive mask value (exp -> exactly 0 in fp32)

FP32 = mybir.dt.float32
DT = mybir.dt.bfloat16


def _split_drain_tile_context():
    """TileContext whose final drain splits its semaphore waits across
    multiple drain instructions — walrus in this container rejects CTRL
    instructions carrying more than one sync wait."""
    import bass_rust

    class SplitDrainTC(tile.TileContext):
        def _drain_and_barrier(self, tick_clock, wait_clock):
            drain_inst = self.nc.sync.drain()
            wait_clock.add_sem_waits(
                drain_inst.ins, tile.ScopedClock({None: tick_clock.global_clock})
            )
            si = drain_inst.ins.sync_info
            if si is not None and si.on_wait and len(si.on_wait) > 1:
                waits = list(si.on_wait)
                si.on_wait = waits[:1]
                drain_inst.ins.sync_info = si
                for w in waits[1:]:
                    d2 = self.nc.sync.drain()
                    d2.ins.sync_info = bass_rust.SyncInfo(on_wait=[w], on_update=[])

            self.nc.all_engine_barrier()
            assert self.sems is not None
            popped = self.nc._tile_sem_poison_stack.pop()
            assert popped is self._sem_poison
            self.nc.clear_and_free_semaphores(list(self.sems.allocated().values()))
            self.nc.all_engine_barrier()

    return SplitDrainTC


def build_graph(nk_per_qc):
    """Build the single-core SPMD graph. nk_per_qc[qc] = number of 128-wide
    key chunks to process for query chunk qc (derived from the mask)."""
    nc = bass.Bass("TRN2", target_bir_lowering=False, debug=False,
                   num_devices=N_CORES)

    hqT = nc.dram_tensor("hqT", [D, LQ], DT, kind="ExternalInput").ap()
    hkvT = nc.dram_tensor("hkvT", [D, LK], DT, kind="ExternalInput").ap()
    wq = nc.dram_tensor("wq", [D, NH * DH], DT, kind="ExternalInput").ap()
    wk = nc.dram_tensor("wk", [D, DH], DT, kind="ExternalInput").ap()
    wv = nc.dram_tensor("wv", [D, DH], DT, kind="ExternalInput").ap()
    wo = nc.dram_tensor("wo", [NH * DH, D], DT, kind="ExternalInput").ap()
    biasT = nc.dram_tensor("biasT", [NH, LK, LQ], DT, kind="ExternalInput").ap()
    out = nc.dram_tensor("out", [LQ, D], DT, kind="ExternalOutput").ap()

    n_s = LQ // QC        # 4 query chunks of 512
    n_m = LQ // 128       # 16 seq chunks of 128

    TC = _split_drain_tile_context()
    with TC(nc) as tc:
        with tc.tile_pool(name="weights", bufs=1) as wpool, \
             tc.tile_pool(name="persist", bufs=1) as ppool:
            ones_sb = wpool.tile([128, 128], DT)
            nc.vector.memset(ones_sb[:], 1.0)
            ident_sb = wpool.tile([128, 128], DT)
            make_identity(nc, ident_sb[:])
            # preload the exp table set while projections run
            warm_sb = wpool.tile([128, 1], FP32)
            nc.scalar.activation(out=warm_sb[:], in_=ones_sb[:, 0:1],
                                 func=mybir.ActivationFunctionType.Exp)

            # persistent activations
            wo_sb = ppool.tile([128, NH, D], DT)      # [hd_in, h, d_out]
            qT_sb = ppool.tile([128, NH, LQ], DT)     # [dh, h, q]
            kT_sb = ppool.tile([128, LK], DT)         # [dh, k]
            v_sb = ppool.tile([128, LK // 128, DH], DT)   # [k_in, k_blk, dh]
            un_sb = ppool.tile([128, NH, LQ], DT)     # normalized U_T

            # ---------------- stage 1: projections ----------------
            with tc.tile_pool(name="w1", bufs=1) as w1pool, \
                 tc.tile_pool(name="slab", bufs=2) as slab_pool, \
                 tc.tile_pool(name="proj_ps", bufs=2, space="PSUM") as proj_ps, \
                 tc.tile_pool(name="vtr_ps", bufs=2, space="PSUM") as vtr_ps, \
                 nc.named_scope("proj"):
                # kv-side weights first (HWDGE ring) so the first matmuls
                # can start as soon as the first hkvT slab lands
                wk_sb = w1pool.tile([128, KO, DH], DT)
                nc.sync.dma_start(
                    out=wk_sb[:], in_=wk.rearrange("(ko p) d -> p ko d", p=128))
                wv_sb = w1pool.tile([128, KO, DH], DT)
                nc.sync.dma_start(
                    out=wv_sb[:], in_=wv.rearrange("(ko p) d -> p ko d", p=128))
                vT_sb = w1pool.tile([128, LK], DT)        # [dh, k]
                wq_sb = w1pool.tile([128, KO, NH * DH], DT)
                # kv side: half-size slabs (16 of 32 ko chunks each) to fit
                # SBUF; kT/vT psums accumulate across both halves
                for s in range(n_s):
                    kt_ps = proj_ps.tile([128, QC], FP32, tag="pps",
                                         name=f"ktps_{s}")
                    vt_ps = proj_ps.tile([128, QC], FP32, tag="pps2",
                                         name=f"vtps_{s}")
                    for half in range(2):
                        slab = slab_pool.tile([128, KO // 2, QC], DT,
                                              tag="slab", name=f"kvslab_{s}_{half}")
                        for g in range(2):
                            r0 = half * 2048 + g * 1024
                            nc.gpsimd.dma_start(
                                out=slab[:, g * 8:(g + 1) * 8, :],
                                in_=hkvT[r0:r0 + 1024,
                                         s * QC:(s + 1) * QC].rearrange(
                                    "(ko p) q -> p ko q", p=128))
                        for kl in range(KO // 2):
                            ko = half * (KO // 2) + kl
                            nc.tensor.matmul(kt_ps[:], lhsT=wk_sb[:, ko, :],
                                             rhs=slab[:, kl, :],
                                             start=(ko == 0), stop=(ko == KO - 1))
                        for kl in range(KO // 2):
                            ko = half * (KO // 2) + kl
                            nc.tensor.matmul(vt_ps[:], lhsT=wv_sb[:, ko, :],
                                             rhs=slab[:, kl, :],
                                             start=(ko == 0), stop=(ko == KO - 1))
                    nc.scalar.copy(out=kT_sb[:, s * QC:(s + 1) * QC], in_=kt_ps[:])
                    nc.scalar.copy(out=vT_sb[:, s * QC:(s + 1) * QC], in_=vt_ps[:])
                    if s == 1:
                        # wq rides the same SWDGE ring mid-phase, landing
                        # well before the q-side matmuls need it
                        for g in range(4):
                            nc.gpsimd.dma_start(
                                out=wq_sb[:, g * 8:(g + 1) * 8, :],
                                in_=wq[g * 1024:(g + 1) * 1024, :].rearrange(
                                    "(ko p) d -> p ko d", p=128))
                # v natural layout via PE transpose of vT
                for blk in range(LK // 128):
                    tp = vtr_ps.tile([128, 128], DT, tag="vtr")
                    nc.tensor.transpose(
                        tp[:], vT_sb[:, blk * 128:(blk + 1) * 128], ident_sb[:])
                    nc.scalar.copy(out=v_sb[:, blk, :], in_=tp[:])
                # q side
                for s in range(n_s):
                    q_pss = [proj_ps.tile([128, QC], FP32,
                                          tag=("pps" if h % 2 == 0 else "pps2"),
                                          name=f"qps_{s}_{h}")
                             for h in range(NH)]
                    for half in range(2):
                        slab = slab_pool.tile([128, KO // 2, QC], DT,
                                              tag="qslab", name=f"qslab_{s}_{half}")
                        for g in range(2):
                            r0 = half * 2048 + g * 1024
                            nc.gpsimd.dma_start(
                                out=slab[:, g * 8:(g + 1) * 8, :],
                                in_=hqT[r0:r0 + 1024,
                                        s * QC:(s + 1) * QC].rearrange(
                                    "(ko p) q -> p ko q", p=128))
                        for h in range(NH):
                            for kl in range(KO // 2):
                                ko = half * (KO // 2) + kl
                                nc.tensor.matmul(
                                    q_pss[h][:],
                                    lhsT=wq_sb[:, ko, h * DH:(h + 1) * DH],
                                    rhs=slab[:, kl, :],
                                    start=(ko == 0), stop=(ko == KO - 1))
                    for h in range(NH):
                        nc.scalar.copy(out=qT_sb[:, h, s * QC:(s + 1) * QC],
                                       in_=q_pss[h][:])
                    if s == 1:
                        for h in range(NH):
                            nc.gpsimd.dma_start(
                                out=wo_sb[:, h, :],
                                in_=wo[h * 128:(h + 1) * 128, :].rearrange(
                                    "(hh p) d -> p hh d", p=128))

            # ------- stage 2+3: attention interleaved with out-proj -------
            # unnormalized U_T and row-sums staged to SBUF by ACT so the
            # reciprocal/normalize never sits on the DVE critical path

            inv2 = float(DH ** -0.5)
            with tc.tile_pool(name="bias", bufs=4) as bias_pool, \
                 tc.tile_pool(name="att_sb", bufs=4) as att_sb, \
                 tc.tile_pool(name="us_sb", bufs=6) as us_pool, \
                 tc.tile_pool(name="sc_ps", bufs=2, space="PSUM") as sc_pool, \
                 tc.tile_pool(name="acc_ps", bufs=2, space="PSUM") as acc_pool, \
                 tc.tile_pool(name="osb", bufs=2) as out_pool, \
                 tc.tile_pool(name="ops", bufs=2, space="PSUM") as out_ps, \
                 nc.named_scope("attn_wo"):

                def emit_attn_unit(qc, h):
                    nk = nk_per_qc[qc]
                    bias_sb = bias_pool.tile([128, LK // 128, QC], DT,
                                             tag="bias", name=f"bias_{qc}_{h}")
                    for g in range((nk + 3) // 4):
                        k0, k1 = g * 4, min(nk, g * 4 + 4)
                        nc.gpsimd.dma_start(
                            out=bias_sb[:, k0:k1, :],
                            in_=biasT[h, k0 * 128:k1 * 128,
                                      qc * QC:(qc + 1) * QC].rearrange(
                                "(ko p) q -> p ko q", p=128))
                    u_ps = acc_pool.tile([128, QC], FP32, tag="ups",
                                         name=f"ups_{qc}_{h}")
                    s_ps = acc_pool.tile([128, QC], FP32, tag="sps",
                                         name=f"sps_{qc}_{h}")
                    e_tiles = {}
                    # software pipeline: score/bias/exp for kc emitted before
                    # the accumulation matmuls of kc-1
                    for kc in range(nk + 1):
                        if kc < nk:
                            sc_ps = sc_pool.tile([128, QC], FP32, tag="scps",
                                                 name=f"scps_{qc}_{h}_{kc}")
                            nc.tensor.matmul(
                                sc_ps[:],
                                lhsT=kT_sb[:, kc * 128:(kc + 1) * 128],
                                rhs=qT_sb[:, h, qc * QC:(qc + 1) * QC],
                                start=True, stop=True)
                            t_sb = att_sb.tile([128, QC], FP32, tag="tsb",
                                               name=f"tsb_{qc}_{h}_{kc}")
                            # t = score * dh^-0.5 + bias
                            nc.vector.scalar_tensor_tensor(
                                out=t_sb[:], in0=sc_ps[:], scalar=inv2,
                                in1=bias_sb[:, kc, :],
                                op0=AluOpType.mult,
                                op1=AluOpType.add)
                            e_sb = att_sb.tile([128, QC], DT, tag="esb",
                                               name=f"esb_{qc}_{h}_{kc}")
                            nc.scalar.activation(
                                out=e_sb[:], in_=t_sb[:],
                                func=mybir.ActivationFunctionType.Exp)
                            e_tiles[kc] = e_sb
                        if kc >= 1:
                            e_prev = e_tiles.pop(kc - 1)
                            nc.tensor.matmul(u_ps[:], lhsT=v_sb[:, kc - 1, :],
                                             rhs=e_prev[:],
                                             start=(kc == 1), stop=(kc == nk))
                            nc.tensor.matmul(s_ps[:], lhsT=ones_sb[:],
                                             rhs=e_prev[:],
                                             start=(kc == 1), stop=(kc == nk))
                    u_sb = us_pool.tile([128, QC], DT, tag="usb",
                                        name=f"usb_{qc}_{h}")
                    s_sb = us_pool.tile([128, QC], DT, tag="ssb",
                                        name=f"ssb_{qc}_{h}")
                    nc.scalar.copy(out=u_sb[:], in_=u_ps[:])
                    nc.scalar.copy(out=s_sb[:], in_=s_ps[:])
                    us_tiles[(qc, h)] = (u_sb, s_sb)

                def emit_norm_piece(qc, h):
                    qsl = slice(qc * QC, (qc + 1) * QC)
                    u_sb, s_sb = us_tiles.pop((qc, h))
                    sinv_sb = att_sb.tile([128, QC], DT, tag="sinv",
                                          name=f"sinv_{qc}_{h}")
                    with nc.allow_low_precision(
                            reason="softmax denom, bf16 ulp is plenty"):
                        nc.vector.reciprocal(out=sinv_sb[:], in_=s_sb[:])
                    nc.vector.tensor_mul(
                        un_sb[:, h, qsl], u_sb[:], sinv_sb[:])

                def emit_wo_unit(m):
                    o_sb = out_pool.tile([128, D], DT, tag="osb",
                                         name=f"osb_{m}")
                    for n in range(D // QC):
                        o_ps = out_ps.tile([128, QC], FP32, tag="ops",
                                           name=f"ops_{m}_{n}")
                        for h in range(NH):
                            nc.tensor.matmul(
                                o_ps[:],
                                lhsT=un_sb[:, h, m * 128:(m + 1) * 128],
                                rhs=wo_sb[:, h, n * QC:(n + 1) * QC],
                                start=(h == 0), stop=(h == NH - 1))
                        nc.scalar.copy(out=o_sb[:, n * QC:(n + 1) * QC],
                                       in_=o_ps[:])
                    nc.scalar.dma_start(out=out[m * 128:(m + 1) * 128, :],
                                        in_=o_sb[:])

                # interleave: attention units stream; norm pieces for qc are
                # woven between the attention units of qc+1 (keeps the
                # reciprocal off the DVE critical path); wo chunks for qc
                # follow once its norm pieces are all emitted
                us_tiles = {}
                wo_queue = []
                for h in range(NH):
                    emit_attn_unit(0, h)
                for qc in range(1, n_s):
                    for h in range(NH):
                        emit_attn_unit(qc, h)
                        emit_norm_piece(qc - 1, h)
                        if h == NH - 1:
                            wo_queue.extend(range(4 * (qc - 1), 4 * qc))
                        if wo_queue:
                            emit_wo_unit(wo_queue.pop(0))
                for h in range(NH):
                    emit_norm_piece(n_s - 1, h)
                    if wo_queue:
                        emit_wo_unit(wo_queue.pop(0))
                wo_queue.extend(range(4 * (n_s - 1), 4 * n_s))
                for m in wo_queue:
                    emit_wo_unit(m)
    _split_waits(nc)
    return nc


def _split_waits(nc):
    """Walrus in this container accepts at most one sync wait per
    instruction: hoist extra waits onto same-engine nops placed directly
    before the instruction (identical semantics — the engine stream
    blocks on each in order)."""
    import bass_rust
    ctr = 0
    for f in nc.m.functions:
        for bb in f.blocks:
            new = []
            for inst in bb.instructions:
                si = inst.sync_info
                if si is not None and si.on_wait and len(si.on_wait) > 1:
                    waits = list(si.on_wait)
                    for w in waits[:-1]:
                        nop = bass_rust.InstNoOp(name=f"waitnop-{ctr}",
                                                 engine=inst.engine)
                        ctr += 1
                        nop.sync_info = bass_rust.SyncInfo(on_wait=[w],
                                                           on_update=[])
                        new.append(nop)
                    si.on_wait = waits[-1:]
                    inst.sync_info = si
                new.append(inst)
            bb.instructions = new


_CACHE = {}


def kernel(hidden_q, hidden_kv, attention_mask, position_bias, Wq, Wk, Wv, Wo):
    hq = np.asarray(hidden_q, dtype=np.float32)[0]      # [2048, 4096]
    hkv = np.asarray(hidden_kv, dtype=np.float32)[0]
    mask = np.asarray(attention_mask)[0]                # [2048, 2048] bool
    pb = np.asarray(position_bias, dtype=np.float32)    # [32, 2048, 2048]
    Wq = np.asarray(Wq, dtype=np.float32)
    Wk = np.asarray(Wk, dtype=np.float32)
    Wv = np.asarray(Wv, dtype=np.float32)
    Wo = np.asarray(Wo, dtype=np.float32)

    # additive mask, transposed to [k, q]
    negT = np.where(mask, np.float32(0.0), np.float32(NEG)).T

    # which 128-key chunks are live for each 512-query chunk
    n_s = LQ // QC
    nk_per_qc = []
    for qc in range(n_s):
        cols = negT[:, qc * QC:(qc + 1) * QC]            # [2048k, 512q]
        live = 0
        for kc in range(LK // 128):
            if np.any(cols[kc * 128:(kc + 1) * 128] != np.float32(NEG)):
                live = kc + 1
        nk_per_qc.append(live)
    key = tuple(nk_per_qc)

    if key not in _CACHE:
        _CACHE[key] = build_graph(nk_per_qc)
    nc = _CACHE[key]

    hqT = np.ascontiguousarray(hq.T).astype(BF16)        # [4096, 2048]
    hkvT = np.ascontiguousarray(hkv.T).astype(BF16)

    in_maps = []
    for i in range(N_CORES):
        bT = np.transpose(pb[NH * i:NH * (i + 1)], (0, 2, 1))  # [4, k, q]
        biasT = (bT + negT[None]).astype(BF16)
        in_maps.append({
            "hqT": hqT,
            "hkvT": hkvT,
            "wq": np.ascontiguousarray(Wq[:, i * NH * DH:(i + 1) * NH * DH]).astype(BF16),
            "wk": np.ascontiguousarray(Wk[:, i * DH:(i + 1) * DH]).astype(BF16),
            "wv": np.ascontiguousarray(Wv[:, i * DH:(i + 1) * DH]).astype(BF16),
            "wo": np.ascontiguousarray(Wo[i * NH * DH:(i + 1) * NH * DH, :]).astype(BF16),
            "biasT": biasT,
        })

    res = run_bass_kernel_spmd(nc, in_maps, list(range(N_CORES)))
    kernel.last_results = res

    acc = np.zeros((LQ, D), dtype=np.float32)
    for i in range(N_CORES):
        acc += res.results[i]["out"].astype(np.float32)
    return acc[None]
